# revision 7
# baseline (speedup 1.0000x reference)
"""GNN message-passing layer (GCN w/ edge-feature attention) on 8 trn2 cores.

Math (per graph b, N=512 nodes, E=8 edge feats, D=64):
    pre_sup = x_b @ W                                   [N, D]
    s[i,j]  = sum_e coef[e] * edge[b,i,j,e]             [N, N]
    adj     = softmax(s / tau, axis=-1)   (tau = 1.0)
    adj_hat = adj + I;  d = rowsum(adj_hat) = 2 exactly (softmax rows sum to 1)
    out     = relu(0.5 * adj_hat @ pre_sup)
            = relu( (P @ (0.5*pre_sup)) / Z + 0.5*pre_sup )
  where P = exp(s) (unnormalized, no max-subtraction needed: |s| <~ 25),
  Z_i = sum_j P[i,j] obtained for free as an extra ones-column in the
  aggregation matmul.

Device mapping (per core: 8 graphs, 64 MiB of edge data = the roofline):
  - scores: 8 PSUM-accumulated PE matmuls per [128, 512*8] edge tile with
    lhsT = coef[e]*I_128 (precomputed on host), rhs = stride-8 e-slice.
  - exp: ACT engine, PSUM -> SBUF.
  - transpose P tiles on PE (is_transpose matmul vs identity), copy to SBUF
    on DVE, then aggregation matmuls contract j with rhs=[0.5*pre_sup | 1].
  - finals: reciprocal + per-partition scale + skip add + relu, DMA out.
"""

import os
from contextlib import ExitStack

import numpy as np

import concourse.bass as bass
import concourse.tile as tile
from concourse import bacc, mybir
from concourse.bass_utils import run_bass_kernel_spmd

F32 = mybir.dt.float32

B, N, E, D = 64, 512, 8, 64
NCORES = 8
BPC = B // NCORES          # graphs per core
PT = 128                   # partition tile (i-rows per edge tile)
NIT = N // PT              # 4 i-tiles (and j-chunks) per graph
TAU = 1.0

# Module-level knobs (test.py pokes these)
TRACE = os.environ.get("KERNEL_TRACE", "") == "1"
LAST_RESULT = None

_nc_cache = {}


def _build_kernel(ctx: ExitStack, tc: "tile.TileContext", edge, x, w, cdiag, ident, out):
    nc = tc.nc

    consts = ctx.enter_context(tc.tile_pool(name="consts", bufs=1))
    edge_pool = ctx.enter_context(tc.tile_pool(name="edge", bufs=4))
    xt_pool = ctx.enter_context(tc.tile_pool(name="xt", bufs=2))
    xT_pool = ctx.enter_context(tc.tile_pool(name="xT", bufs=2))
    psup_pool = ctx.enter_context(tc.tile_pool(name="psup", bufs=2))
    p_pool = ctx.enter_context(tc.tile_pool(name="p", bufs=2))
    pT_pool = ctx.enter_context(tc.tile_pool(name="pT", bufs=2))
    fin_pool = ctx.enter_context(tc.tile_pool(name="fin", bufs=3))
    o_pool = ctx.enter_context(tc.tile_pool(name="o", bufs=3))

    misc_psum = ctx.enter_context(tc.tile_pool(name="mpsum", bufs=2, space="PSUM"))
    sc_psum = ctx.enter_context(tc.tile_pool(name="scpsum", bufs=2, space="PSUM"))
    pT_psum = ctx.enter_context(tc.tile_pool(name="ptpsum", bufs=2, space="PSUM"))
    out_psum = ctx.enter_context(tc.tile_pool(name="opsum", bufs=2, space="PSUM"))

    # Constants
    cd = consts.tile([PT, E * PT], F32)       # cd[:, e*128:(e+1)*128] = coef[e] * I
    nc.sync.dma_start(cd[:], cdiag[:])
    idn = consts.tile([PT, PT], F32)
    nc.sync.dma_start(idn[:], ident[:])
    wsb = consts.tile([D, D], F32)
    nc.sync.dma_start(wsb[:], w[:])

    x_r = x[:].rearrange("(b it p) d -> b p it d", b=BPC, it=NIT, p=PT)

    def compute_psup(b):
        """pre_sup' = 0.5 * (x_b @ W) with a trailing ones column per j-chunk."""
        xt = xt_pool.tile([PT, NIT * D], F32)
        nc.sync.dma_start(xt[:].rearrange("p (it d) -> p it d", it=NIT), x_r[b])
        psup = psup_pool.tile([PT, NIT * (D + 1)], F32)
        for it in range(NIT):
            xT_ps = misc_psum.tile([D, PT], F32, tag="m")
            nc.tensor.matmul(xT_ps[:], xt[:, it * D:(it + 1) * D], idn[:],
                             is_transpose=True)
            xT_sb = xT_pool.tile([D, PT], F32)
            nc.vector.tensor_copy(xT_sb[:], xT_ps[:])
            ps_ps = misc_psum.tile([PT, D], F32, tag="m")
            nc.tensor.matmul(ps_ps[:], xT_sb[:], wsb[:], start=True, stop=True)
            nc.scalar.mul(psup[:, it * (D + 1):it * (D + 1) + D], ps_ps[:], 0.5)
            nc.vector.memset(psup[:, it * (D + 1) + D:(it + 1) * (D + 1)], 1.0)
        return psup

    def scores_tile(b, it):
        """DMA one edge tile and run the 8 accumulating score matmuls."""
        et = edge_pool.tile([PT, N * E], F32)
        row0 = b * N + it * PT
        nc.sync.dma_start(et[:], edge[row0:row0 + PT, :])
        et3 = et[:].rearrange("p (j e) -> p j e", e=E)
        sc_ps = sc_psum.tile([PT, N], F32)
        for e in range(E):
            nc.tensor.matmul(sc_ps[:], cd[:, e * PT:(e + 1) * PT], et3[:, :, e],
                             start=(e == 0), stop=(e == E - 1))
        p_sb = p_pool.tile([PT, N], F32)
        nc.scalar.activation(p_sb[:], sc_ps[:],
                             mybir.ActivationFunctionType.Exp, scale=1.0 / TAU)
        return p_sb

    def post_tile(b, it, p_sb, psup):
        """Transpose P, aggregate against pre_sup'+ones, normalize, relu, store."""
        pT_sb = pT_pool.tile([PT, N], F32)
        for jc in range(NIT):
            pT_ps = pT_psum.tile([PT, PT], F32)
            nc.tensor.matmul(pT_ps[:], p_sb[:, jc * PT:(jc + 1) * PT], idn[:],
                             is_transpose=True)
            nc.vector.tensor_copy(pT_sb[:, jc * PT:(jc + 1) * PT], pT_ps[:])
        o_ps = out_psum.tile([PT, D + 1], F32)
        for jc in range(NIT):
            nc.tensor.matmul(o_ps[:], pT_sb[:, jc * PT:(jc + 1) * PT],
                             psup[:, jc * (D + 1):(jc + 1) * (D + 1)],
                             start=(jc == 0), stop=(jc == NIT - 1))
        r = fin_pool.tile([PT, 1], F32, tag="r")
        nc.vector.reciprocal(r[:], o_ps[:, D:D + 1])
        t1 = fin_pool.tile([PT, D], F32, tag="t1")
        nc.vector.tensor_scalar_mul(t1[:], o_ps[:, 0:D], r[:])
        t2 = fin_pool.tile([PT, D], F32, tag="t2")
        nc.vector.tensor_add(t2[:], t1[:],
                             psup[:, it * (D + 1):it * (D + 1) + D])
        o_sb = o_pool.tile([PT, D], F32)
        nc.scalar.activation(o_sb[:], t2[:], mybir.ActivationFunctionType.Relu)
        row0 = b * N + it * PT
        nc.sync.dma_start(out[row0:row0 + PT, :], o_sb[:])

    # Software-pipelined emission: post(k-1) lands between scores(k) and
    # scores(k+1) so the PE never waits on ACT's exp.
    pending = None
    for b in range(BPC):
        psup = compute_psup(b)
        for it in range(NIT):
            p_sb = scores_tile(b, it)
            if pending is not None:
                post_tile(*pending)
            pending = (b, it, p_sb, psup)
    post_tile(*pending)


def _get_nc(reps: int = 1):
    key = f"v1-r{reps}"
    if key not in _nc_cache:
        nc = bacc.Bacc("TRN2", target_bir_lowering=False, debug=False,
                       num_devices=NCORES)
        edge = nc.declare_dram_parameter("edge", [BPC * N, N * E], F32, isOutput=False)
        x = nc.declare_dram_parameter("x", [BPC * N, D], F32, isOutput=False)
        w = nc.declare_dram_parameter("w", [D, D], F32, isOutput=False)
        cdiag = nc.declare_dram_parameter("cdiag", [PT, E * PT], F32, isOutput=False)
        ident = nc.declare_dram_parameter("ident", [PT, PT], F32, isOutput=False)
        out = nc.declare_dram_parameter("out", [BPC * N, D], F32, isOutput=True)
        with tile.TileContext(nc) as tc:
            for _ in range(reps):
                with ExitStack() as ctx:
                    _build_kernel(ctx, tc, edge, x, w, cdiag, ident, out)
        nc.compile()
        _nc_cache[key] = nc
    return _nc_cache[key]


def kernel(**inputs) -> np.ndarray:
    global LAST_RESULT
    edge = np.ascontiguousarray(inputs["edge_features"], dtype=np.float32)
    x = np.ascontiguousarray(inputs["x"], dtype=np.float32)
    W = np.ascontiguousarray(inputs["W"], dtype=np.float32)
    coef = np.asarray(inputs["coef"], dtype=np.float32)

    c = coef[:, 0]
    cdiag = np.zeros((PT, E * PT), np.float32)
    ar = np.arange(PT)
    for e in range(E):
        cdiag[ar, e * PT + ar] = c[e]
    ident = np.eye(PT, dtype=np.float32)

    nc = _get_nc()
    in_maps = []
    for core in range(NCORES):
        b0 = core * BPC
        in_maps.append({
            "edge": edge[b0:b0 + BPC].reshape(BPC * N, N * E),
            "x": x[b0 * N:(b0 + BPC) * N],
            "w": W,
            "cdiag": cdiag,
            "ident": ident,
        })
    res = run_bass_kernel_spmd(nc, in_maps, list(range(NCORES)), trace=TRACE)
    LAST_RESULT = res
    return np.concatenate([res.results[i]["out"] for i in range(NCORES)], axis=0)


# revision 8
# speedup vs baseline: 86.8121x; 86.8121x over previous
"""GNN message-passing layer (GCN w/ edge-feature attention) on 8 trn2 cores.

Math (per graph b, N=512 nodes, E=8 edge feats, D=64):
    pre_sup = x_b @ W                                   [N, D]
    s[i,j]  = sum_e coef[e] * edge[b,i,j,e]             [N, N]
    adj     = softmax(s / tau, axis=-1)   (tau = 1.0)
    adj_hat = adj + I;  d = rowsum(adj_hat) = 2 exactly (softmax rows sum to 1)
    out     = relu(0.5 * adj_hat @ pre_sup)
            = relu( (P @ (0.5*pre_sup)) / Z + 0.5*pre_sup )
  where P = exp(s) (unnormalized, no max-subtraction needed: |s| <~ 25),
  Z_i = sum_j P[i,j] obtained for free as an extra ones-column in the
  aggregation matmul.

Device mapping (per core: 8 graphs, 64 MiB of edge data = the roofline):
  - scores: 8 PSUM-accumulated PE matmuls per [128, 512*8] edge tile with
    lhsT = coef[e]*I_128 (precomputed on host), rhs = stride-8 e-slice.
  - exp: ACT engine, PSUM -> SBUF.
  - transpose P tiles on PE (is_transpose matmul vs identity), copy to SBUF
    on DVE, then aggregation matmuls contract j with rhs=[0.5*pre_sup | 1].
  - finals: reciprocal + per-partition scale + skip add + relu, DMA out.
"""

import os
from contextlib import ExitStack

import numpy as np

import concourse.bass as bass
import concourse.tile as tile
from concourse import bacc, mybir
from concourse.bass_utils import run_bass_kernel_spmd

F32 = mybir.dt.float32

B, N, E, D = 64, 512, 8, 64
NCORES = 8
BPC = B // NCORES          # graphs per core
PT = 128                   # partition tile (i-rows per edge tile)
NIT = N // PT              # 4 i-tiles (and j-chunks) per graph
TAU = 1.0

# Module-level knobs (test.py pokes these)
TRACE = os.environ.get("KERNEL_TRACE", "") == "1"
LAST_RESULT = None

_nc_cache = {}


def _build_kernel(ctx: ExitStack, tc: "tile.TileContext", edge, x, w, cdiag, ident, out):
    nc = tc.nc

    consts = ctx.enter_context(tc.tile_pool(name="consts", bufs=1))
    edge_pool = ctx.enter_context(tc.tile_pool(name="edge", bufs=4))
    xt_pool = ctx.enter_context(tc.tile_pool(name="xt", bufs=2))
    xT_pool = ctx.enter_context(tc.tile_pool(name="xT", bufs=2))
    psup_pool = ctx.enter_context(tc.tile_pool(name="psup", bufs=2))
    p_pool = ctx.enter_context(tc.tile_pool(name="p", bufs=2))
    pT_pool = ctx.enter_context(tc.tile_pool(name="pT", bufs=2))
    fin_pool = ctx.enter_context(tc.tile_pool(name="fin", bufs=3))
    o_pool = ctx.enter_context(tc.tile_pool(name="o", bufs=3))

    misc_psum = ctx.enter_context(tc.tile_pool(name="mpsum", bufs=2, space="PSUM"))
    sc_psum = ctx.enter_context(tc.tile_pool(name="scpsum", bufs=2, space="PSUM"))
    pT_psum = ctx.enter_context(tc.tile_pool(name="ptpsum", bufs=2, space="PSUM"))
    out_psum = ctx.enter_context(tc.tile_pool(name="opsum", bufs=2, space="PSUM"))

    # Constants
    cd = consts.tile([PT, E * PT], F32)       # cd[:, e*128:(e+1)*128] = coef[e] * I
    nc.sync.dma_start(cd[:], cdiag[:])
    idn = consts.tile([PT, PT], F32)
    nc.sync.dma_start(idn[:], ident[:])
    wsb = consts.tile([D, D], F32)
    nc.sync.dma_start(wsb[:], w[:])

    x_r = x[:].rearrange("(b it p) d -> b p it d", b=BPC, it=NIT, p=PT)

    def compute_psup(b):
        """pre_sup' = 0.5 * (x_b @ W) with a trailing ones column per j-chunk."""
        xt = xt_pool.tile([PT, NIT * D], F32)
        nc.sync.dma_start(xt[:].rearrange("p (it d) -> p it d", it=NIT), x_r[b])
        psup = psup_pool.tile([PT, NIT * (D + 1)], F32)
        for it in range(NIT):
            xT_ps = misc_psum.tile([D, PT], F32, tag="m")
            nc.tensor.matmul(xT_ps[:], xt[:, it * D:(it + 1) * D], idn[:],
                             is_transpose=True)
            xT_sb = xT_pool.tile([D, PT], F32)
            nc.vector.tensor_copy(xT_sb[:], xT_ps[:])
            ps_ps = misc_psum.tile([PT, D], F32, tag="m")
            nc.tensor.matmul(ps_ps[:], xT_sb[:], wsb[:], start=True, stop=True)
            nc.scalar.mul(psup[:, it * (D + 1):it * (D + 1) + D], ps_ps[:], 0.5)
            nc.vector.memset(psup[:, it * (D + 1) + D:(it + 1) * (D + 1)], 1.0)
        return psup

    def scores_tile(b, it):
        """DMA one edge tile and run the 8 accumulating score matmuls."""
        et = edge_pool.tile([PT, N * E], F32)
        row0 = b * N + it * PT
        nc.sync.dma_start(et[:], edge[row0:row0 + PT, :])
        et3 = et[:].rearrange("p (j e) -> p j e", e=E)
        sc_ps = sc_psum.tile([PT, N], F32)
        for e in range(E):
            nc.tensor.matmul(sc_ps[:], cd[:, e * PT:(e + 1) * PT], et3[:, :, e],
                             start=(e == 0), stop=(e == E - 1))
        p_sb = p_pool.tile([PT, N], F32)
        nc.scalar.activation(p_sb[:], sc_ps[:],
                             mybir.ActivationFunctionType.Exp, scale=1.0 / TAU)
        return p_sb

    def post_tile(b, it, p_sb, psup):
        """Transpose P, aggregate against pre_sup'+ones, normalize, relu, store."""
        pT_sb = pT_pool.tile([PT, N], F32)
        for jc in range(NIT):
            pT_ps = pT_psum.tile([PT, PT], F32)
            nc.tensor.matmul(pT_ps[:], p_sb[:, jc * PT:(jc + 1) * PT], idn[:],
                             is_transpose=True)
            nc.vector.tensor_copy(pT_sb[:, jc * PT:(jc + 1) * PT], pT_ps[:])
        o_ps = out_psum.tile([PT, D + 1], F32)
        for jc in range(NIT):
            nc.tensor.matmul(o_ps[:], pT_sb[:, jc * PT:(jc + 1) * PT],
                             psup[:, jc * (D + 1):(jc + 1) * (D + 1)],
                             start=(jc == 0), stop=(jc == NIT - 1))
        r = fin_pool.tile([PT, 1], F32, tag="r")
        nc.vector.reciprocal(r[:], o_ps[:, D:D + 1])
        t1 = fin_pool.tile([PT, D], F32, tag="t1")
        nc.vector.tensor_scalar_mul(t1[:], o_ps[:, 0:D], r[:])
        t2 = fin_pool.tile([PT, D], F32, tag="t2")
        nc.vector.tensor_add(t2[:], t1[:],
                             psup[:, it * (D + 1):it * (D + 1) + D])
        o_sb = o_pool.tile([PT, D], F32)
        nc.scalar.activation(o_sb[:], t2[:], mybir.ActivationFunctionType.Relu)
        row0 = b * N + it * PT
        nc.sync.dma_start(out[row0:row0 + PT, :], o_sb[:])

    # Software-pipelined emission: post(k-1) lands between scores(k) and
    # scores(k+1) so the PE never waits on ACT's exp.
    pending = None
    for b in range(BPC):
        psup = compute_psup(b)
        for it in range(NIT):
            p_sb = scores_tile(b, it)
            if pending is not None:
                post_tile(*pending)
            pending = (b, it, p_sb, psup)
    post_tile(*pending)


def _build_dma_only(ctx: ExitStack, tc: "tile.TileContext", edge, x, w, cdiag, ident, out):
    """Variant: just the edge DMA stream + a trivial out write (BW probe)."""
    nc = tc.nc
    edge_pool = ctx.enter_context(tc.tile_pool(name="edge", bufs=4))
    o_pool = ctx.enter_context(tc.tile_pool(name="o", bufs=2))
    for b in range(BPC):
        for it in range(NIT):
            et = edge_pool.tile([PT, N * E], F32)
            row0 = b * N + it * PT
            nc.sync.dma_start(et[:], edge[row0:row0 + PT, :])
            o_sb = o_pool.tile([PT, D], F32)
            nc.vector.tensor_copy(o_sb[:], et[:, 0:D])
            nc.sync.dma_start(out[row0:row0 + PT, :], o_sb[:])


_BUILDERS = {"v1": _build_kernel, "dma": _build_dma_only}


def _get_nc(reps: int = 1, variant: str = "v1"):
    key = f"{variant}-r{reps}"
    if key not in _nc_cache:
        nc = bacc.Bacc("TRN2", target_bir_lowering=False, debug=False,
                       num_devices=NCORES)
        edge = nc.declare_dram_parameter("edge", [BPC * N, N * E], F32, isOutput=False)
        x = nc.declare_dram_parameter("x", [BPC * N, D], F32, isOutput=False)
        w = nc.declare_dram_parameter("w", [D, D], F32, isOutput=False)
        cdiag = nc.declare_dram_parameter("cdiag", [PT, E * PT], F32, isOutput=False)
        ident = nc.declare_dram_parameter("ident", [PT, PT], F32, isOutput=False)
        out = nc.declare_dram_parameter("out", [BPC * N, D], F32, isOutput=True)
        builder = _BUILDERS[variant]
        with tile.TileContext(nc) as tc:
            for _ in range(reps):
                with ExitStack() as ctx:
                    builder(ctx, tc, edge, x, w, cdiag, ident, out)
        nc.compile()
        _nc_cache[key] = nc
    return _nc_cache[key]


def kernel(**inputs) -> np.ndarray:
    global LAST_RESULT
    edge = np.ascontiguousarray(inputs["edge_features"], dtype=np.float32)
    x = np.ascontiguousarray(inputs["x"], dtype=np.float32)
    W = np.ascontiguousarray(inputs["W"], dtype=np.float32)
    coef = np.asarray(inputs["coef"], dtype=np.float32)

    c = coef[:, 0]
    cdiag = np.zeros((PT, E * PT), np.float32)
    ar = np.arange(PT)
    for e in range(E):
        cdiag[ar, e * PT + ar] = c[e]
    ident = np.eye(PT, dtype=np.float32)

    nc = _get_nc()
    in_maps = []
    for core in range(NCORES):
        b0 = core * BPC
        in_maps.append({
            "edge": edge[b0:b0 + BPC].reshape(BPC * N, N * E),
            "x": x[b0 * N:(b0 + BPC) * N],
            "w": W,
            "cdiag": cdiag,
            "ident": ident,
        })
    res = run_bass_kernel_spmd(nc, in_maps, list(range(NCORES)), trace=TRACE)
    LAST_RESULT = res
    return np.concatenate([res.results[i]["out"] for i in range(NCORES)], axis=0)
